# revision 23
# baseline (speedup 1.0000x reference)
"""AxialAttention3D Trainium2 kernel (v3).

Reference: 3 branches (d/h/w) of full global MHA over N = 16^3 = 4096
positions of x (1, 128, 16, 16, 16), 8 heads x dim_head 16;
    out = gamma * (out_d + out_h + out_w) + x.
Sharding: core c computes head c of all 3 branches; host sums partials.

The kernel is elementwise-bound: every softmax score crosses PSUM->SBUF
through ACT or DVE (DMA/GPSIMD have no PSUM route).  Design:
  - exp split across BOTH engines: ACT items run native exp
    (scale=0.25); DVE items run the Schraudolph bit-trick - ONE
    tensor_scalar op computing round(A*s + B) into an int16 view of the
    bf16 P tile (HW-verified exact RNE convert; +-3.3% sawtooth that the
    softmax ratio largely cancels).
  - q/k biases fold into a 17th contraction row (k side carries
    r.x = bq.W_k x via an extra lk column; the shared +1 offset from the
    copy's per-partition add is softmax-invariant).  No bias adds.
  - scores: K=17 row-tiled matmuls at 4 band offsets (replicated q/k).
  - attn@V: col-tiled per unit (tile_position=(0,32u)), emitted in
    band-rotating 6-MM blocks (one group of 2 m-tiles x 3 units) kept
    free of interleaved full-width matmuls so the 3 col bands overlap.
  - GRP=2 (1024-wide PSUM score tiles) with bufs=3: scores(i+3) waits
    exp(i) - deep enough that the exp engines never starve.
  - denominators ride as a 17th vT ones-column through attn@V; per chunk
    one 96-row PSUM->SBUF copy + one reciprocal_approx_fast + DMA
    log-doubling broadcast; norm and out-proj are emission-deferred so
    the DVE FIFO never head-of-line blocks on DMA latency.
  - out-proj bias (incl. folded V-bias) rides wo row 96 against a
    constant ones row of `scaled`.
"""

import math

import numpy as np


def _bf16np():
    import ml_dtypes

    return ml_dtypes.bfloat16


HEADS = 8
DH = 16
C = 128
NCORES = 8

A_SCH = 46.16624130844683  # 0.25 * 128 / ln 2
B_SCH = 16249.25
A_SCH8 = 1.4426950408889634  # 0.25 * 4 / ln 2 (e5m2 bits)
B_SCH8 = 60.8125

_FULL = dict(MT=32, CHUNK=512, NCH=8, GRP=2, ACT_FRAC=0.567, LAG=4, EPI_DELAY=3)
_CACHE = {}


def _patch_tile_drain():
    """walrus in this env rejects >1 sync wait on one instruction; split the
    Tile kernel-tail drain's aggregated waits into one drain per wait."""
    import concourse.mybir as mybir
    from concourse.tile import TileContext, ScopedClock

    if getattr(TileContext, "_drain_split_patched", False):
        return

    def _drain_and_barrier_split(self, tick_clock, wait_clock):
        probe = self.nc.sync.drain()
        wait_clock.add_sem_waits(
            probe.ins, ScopedClock({None: tick_clock.global_clock})
        )
        si = probe.ins.sync_info
        waits = list(si.on_wait) if si is not None else []
        if len(waits) > 1:
            si.on_wait = [waits[0]]
            for w in waits[1:]:
                d = self.nc.sync.drain()
                d.ins.sync_info = mybir.SyncInfo(on_wait=[w], on_update=[])
        self.nc.all_engine_barrier()
        assert self.sems is not None
        popped = self.nc._tile_sem_poison_stack.pop()
        assert popped is self._sem_poison
        self.nc.clear_and_free_semaphores(list(self.sems.allocated().values()))
        self.nc.all_engine_barrier()

    TileContext._drain_and_barrier = _drain_and_barrier_split
    TileContext._drain_split_patched = True


def _split_multi_waits(nc):
    """walrus in this env allows at most ONE sync wait per instruction.
    Hoist extra waits onto same-engine NoOps inserted just before."""
    import concourse.mybir as mybir

    for f in nc.m.functions:
        for bb in f.blocks:
            new = []
            changed = False
            for inst in bb.instructions:
                si = inst.sync_info
                if si is not None and si.on_wait and len(si.on_wait) > 1:
                    waits = list(si.on_wait)
                    for j, w in enumerate(waits[:-1]):
                        nop = mybir.InstNoOp(
                            name=f"{inst.name}-w{j}",
                            engine=inst.engine,
                            sync_info=mybir.SyncInfo(on_wait=[w], on_update=[]),
                            bass_nofuse=True,
                        )
                        new.append(nop)
                    si.on_wait = [waits[-1]]
                    changed = True
                new.append(inst)
            if changed:
                bb.instructions = new


def build_nc(cfg=_FULL, split_waits=True):
    import concourse.bass as bass
    import concourse.mybir as mybir
    from concourse import tile

    _patch_tile_drain()

    f32 = mybir.dt.float32
    f32r = mybir.dt.float32r
    bf16 = mybir.dt.bfloat16
    i16 = mybir.dt.int16
    i8 = mybir.dt.int8
    e4 = mybir.dt.float8e4
    e5 = mybir.dt.float8e5
    DR = mybir.MatmulPerfMode.DoubleRow
    Exp = mybir.ActivationFunctionType.Exp
    Ident = mybir.ActivationFunctionType.Identity
    Mult = mybir.AluOpType.mult
    Add = mybir.AluOpType.add

    MT, CHUNK, NCH, GRP = cfg["MT"], cfg["CHUNK"], cfg["NCH"], cfg["GRP"]
    LAG, EPI_DELAY = cfg["LAG"], cfg["EPI_DELAY"]
    N = MT * 128
    assert N == CHUNK * NCH
    TPC = CHUNK // 128  # m-tiles per chunk (4)
    assert MT % GRP == 0
    NGRP = MT // GRP
    per_chunk = 3 * NGRP

    nc = bass.Bass("TRN2", target_bir_lowering=False, debug=False)

    x_d = nc.declare_dram_parameter("x", [C, N], bf16, isOutput=False)
    lq_d = [
        nc.declare_dram_parameter(f"lq{u}", [C, 128], bf16, isOutput=False)
        for u in range(3)
    ]
    lk_d = [
        nc.declare_dram_parameter(f"lk{u}", [C, 128], bf16, isOutput=False)
        for u in range(3)
    ]
    addv_d = nc.declare_dram_parameter("addv", [C, 1], f32, isOutput=False)
    wv_d = nc.declare_dram_parameter("wv3", [C, 52], bf16, isOutput=False)
    wo_d = nc.declare_dram_parameter("wo", [C, 128], f32r, isOutput=False)
    onesr_d = nc.declare_dram_parameter("onesr", [C, CHUNK], f32r, isOutput=False)
    y_d = nc.declare_dram_parameter("y", [C, N], f32, isOutput=True)

    # exp engine schedule: True -> ACT, False -> DVE (Schraudolph)
    n_items = NCH * NGRP * 3
    act_frac = cfg["ACT_FRAC"]
    eng_act = []
    accf = 0.0
    for _ in range(n_items):
        accf += act_frac
        if accf >= 1.0:
            eng_act.append(True)
            accf -= 1.0
        else:
            eng_act.append(False)

    with tile.TileContext(nc) as tc:
        with (
            tc.tile_pool(name="persist", bufs=1) as pp,
            tc.tile_pool(name="dst2", bufs=2) as dstp2,
            tc.tile_pool(name="pt", bufs=16) as ptp,
            tc.tile_pool(name="osb", bufs=2) as osbp,
            tc.tile_pool(name="big", bufs=3, space="PSUM") as bigp,
            tc.tile_pool(name="accp", bufs=1, space="PSUM") as accp,
            tc.tile_pool(name="projp", bufs=1, space="PSUM") as projp,
        ):
            # ---- persistent SBUF tensors ----
            x_sb = pp.tile([C, N], bf16, name="x_sb", tag="x")
            lq = [pp.tile([C, 128], bf16, name=f"lq{u}_sb", tag=f"lq{u}") for u in range(3)]
            lk = [pp.tile([C, 128], bf16, name=f"lk{u}_sb", tag=f"lk{u}") for u in range(3)]
            addv = pp.tile([C, 1], f32, name="addv_sb", tag="addv")
            wv = pp.tile([C, 52], bf16, name="wv_sb", tag="wv")
            wo = pp.tile([C, 128], f32r, name="wo_sb", tag="wo")
            for u in range(3):
                nc.sync.dma_start(lq[u][:], lq_d[u][:])
                nc.sync.dma_start(lk[u][:], lk_d[u][:])
            nc.sync.dma_start(addv[:], addv_d[:])
            nc.sync.dma_start(wv[:], wv_d[:])
            nc.sync.dma_start(wo[:], wo_d[:])
            for cidx in range(NCH):
                # split each chunk across two queues for latency
                h = CHUNK // 2
                for s in range(2):
                    a = cidx * CHUNK + s * h
                    nc.sync.dma_start(x_sb[:, a : a + h], x_d[:, a : a + h])

            # qk[u]: per chunk c, [c*1024, +512) = q, [c*1024+512, +1024) = k
            qk = [pp.tile([C, 2 * N], bf16, name=f"qk{u}_sb", tag=f"qk{u}") for u in range(3)]
            # vT8: fp8e4, zero-padded so every unit's DR matmul writes from
            # partition 0 (DR + col tile_position is an invalid ISA combo).
            # Units pack at 17-row offsets: u rows 17u..17u+16 (last = denom).
            # Per pair (288 B): u0 [j=2 x 32] @0, u1 [j=2 x 48] @64 (data cols
            # 17..33), u2 [j=2 x 64] @160 (data cols 34..50).
            PAIRB = 288
            VOFF = (0, 64, 160)
            VW = (32, 48, 64)
            vT8 = pp.tile([C, (MT // 2) * PAIRB], e4, name="vT8_sb", tag="vT8")
            nc.gpsimd.memset(vT8[:], 0.0)
            vT8q = vT8[:].rearrange("p (q z) -> p q z", z=PAIRB)
            for u in range(3):
                for j in range(2):
                    col = VOFF[u] + j * VW[u] + 17 * u + 16
                    nc.vector.memset(vT8q[:, :, col : col + 1], 1.0)
            denb = pp.tile([C, 16], f32, name="denb_sb", tag="denb")
            recb = pp.tile([C, 16], f32, name="recb_sb", tag="recb")
            normsb = pp.tile([C, CHUNK], f32, name="normsb_sb", tag="normsb")
            nc.gpsimd.memset(normsb[:], 0.0)
            scaled = pp.tile([C, CHUNK], f32r, name="scaled_sb", tag="scaled")
            nc.sync.dma_start(scaled[:], onesr_d[:])

            qk_copy_flip = [True]  # alternate ACT/DVE for qk copies

            def emit_qk(u, cidx):
                cs, ce = cidx * CHUNK, (cidx + 1) * CHUNK
                ps = bigp.tile([C, 2 * CHUNK], f32, name="qkps", tag="scores")
                nc.tensor.matmul(
                    ps[:, 0:CHUNK], lhsT=lq[u][:], rhs=x_sb[:, cs:ce],
                    start=True, stop=True,
                )
                nc.tensor.matmul(
                    ps[:, CHUNK : 2 * CHUNK], lhsT=lk[u][:], rhs=x_sb[:, cs:ce],
                    start=True, stop=True,
                )
                dst = qk[u][:, cidx * 2 * CHUNK : (cidx + 1) * 2 * CHUNK]
                if qk_copy_flip[0]:
                    nc.scalar.activation(dst, ps[:], Ident, bias=addv[:], scale=1.0)
                else:
                    nc.vector.tensor_scalar_add(dst, ps[:], addv[:])
                qk_copy_flip[0] = not qk_copy_flip[0]

            def emit_vt(B):
                # batch of 4 m-tiles (2 DR pairs) -> one strided copy + memset
                ts = [4 * B + i for i in range(4)]
                ps = bigp.tile([C, 51 * 4], f32, name="vps", tag="scores")
                for i, t in enumerate(ts):
                    nc.tensor.matmul(
                        ps[:, i * 51 : (i + 1) * 51],
                        lhsT=x_sb[:, t * 128 : (t + 1) * 128],
                        rhs=wv[:, 0:51],
                        start=True, stop=True,
                    )
                # copy per unit: dst AP [(PAIRB, 2 pairs), (VW_u, 2 j), (1, 16)]
                src3 = ps[:].rearrange("p (q j u d) -> p q j u d", q=2, j=2, u=3)
                for u in range(3):
                    ap = vT8q[:, 2 * B : 2 * B + 2, VOFF[u] : VOFF[u] + 2 * VW[u]]
                    ap = ap.rearrange("p q (j w) -> p q j w", j=2)[
                        :, :, :, 17 * u : 17 * u + 16
                    ]
                    nc.vector.tensor_copy(ap, src3[:, :, :, u, 0:16])

            # ---- item schedule: item = (c, g, u), u fastest ----
            items = [
                (c, g, u) for c in range(NCH) for g in range(NGRP) for u in range(3)
            ]

            # drip deadlines for remaining qk / vt emissions (chunk 0 seeded)
            drip = []
            for ck in range(1, NCH):
                dl = min(ck * per_chunk, 3 * 2 * ck)
                for u in range(3):
                    drip.append((max(0, dl + u - 3), "qk", (u, ck)))
            for B in range(1, MT // 4):
                drip.append((max(0, 6 * B - 3), "vt", B))
            drip.sort(key=lambda z: z[0])

            pt_of_item = {}
            acc_of_chunk = {}
            deferred = []  # (slot, fn)

            def emit_scores(idx):
                c, g, u = items[idx]
                tlist = [GRP * g + i for i in range(GRP)]
                sc = bigp.tile([C, CHUNK * GRP], f32, name="sc_ps", tag="scores")
                qs = c * 2 * CHUNK
                for i, t in enumerate(tlist):
                    # per-unit band phase: consecutive items' score MMs rotate
                    # across all 4 PE row bands, which the PE merges (~135
                    # ns/MM measured vs 213 serial)
                    r = (t + 2 * u) % 4
                    ck, ko = t // TPC, (t % TPC) * 128
                    kbase = ck * 2 * CHUNK + CHUNK + ko
                    nc.tensor.matmul(
                        sc[:, i * CHUNK : (i + 1) * CHUNK],
                        lhsT=qk[u][32 * r : 32 * r + 17, kbase : kbase + 128],
                        rhs=qk[u][32 * r : 32 * r + 17, qs : qs + CHUNK],
                        start=True,
                        stop=True,
                        tile_position=(32 * r, 0),
                    )
                pt = ptp.tile([C, CHUNK * GRP], e5, name="pt_sb", tag="pt")
                # interleaved pair layout: byte offset of (t_local, n) = 2n + t
                pt_ap = pt[:].rearrange("p (n j) -> p j n", j=GRP)
                if eng_act[idx]:
                    nc.scalar.activation(pt_ap, sc[:], Exp, bias=0.0, scale=0.25)
                else:
                    nc.vector.tensor_scalar(
                        pt_ap.bitcast(i8), sc[:], A_SCH8, B_SCH8, op0=Mult, op1=Add
                    )
                pt_of_item[idx] = pt

            def emit_attnv_block(c, g):
                # one group x 3 units, t-major so the 3 col bands rotate
                if c not in acc_of_chunk:
                    acc_of_chunk[c] = accp.tile([C, CHUNK], f32, name="acc_ps", tag="acc")
                acc = acc_of_chunk[c]
                base = (c * NGRP + g) * 3
                pts = [pt_of_item.pop(base + u) for u in range(3)]
                # widest unit first at g==0 so its start=True lays down the
                # has_written bits the narrower units accumulate into
                for u in (2, 1, 0):
                    lb = g * PAIRB + VOFF[u]
                    lhs = vT8[:, lb : lb + 2 * VW[u]].rearrange(
                        "p (j z) -> p j z", j=2
                    )[:, :, 0 : 17 * u + 17]
                    nc.tensor.matmul(
                        acc[0 : 17 * u + 17, :],
                        lhsT=lhs,
                        rhs=pts[u][:].rearrange("p (n j) -> p j n", j=2),
                        start=(g == 0 and u == 2),
                        stop=(g == NGRP - 1 and u == 0),
                        perf_mode=DR,
                    )

            dstage_of_chunk = {}

            def emit_denoms(c):
                # one fast PSUM->SBUF copy frees the acc bank; the slow
                # reciprocal+broadcast chain then runs off the SBUF staging
                # copy without gating the next chunk's attn@V.
                acc = acc_of_chunk.pop(c)
                dstage = dstp2.tile([C, CHUNK], f32, name="dst_sb", tag="dst")
                dstage_of_chunk[c] = dstage
                nc.vector.tensor_copy(dstage[0:64, :], acc[0:64, :])
                for u in range(3):
                    nc.sync.dma_start(
                        denb[32 * u : 32 * u + 32, :],
                        dstage[17 * u + 16 : 17 * u + 17, :],
                    )
                nc.vector.reciprocal(recb[0:96, :], denb[0:96, :])
                for u in range(3):
                    b = 17 * u
                    nc.sync.dma_start(normsb[b : b + 1, :], recb[32 * u : 32 * u + 32, :])
                    for w in (1, 2, 4, 8):
                        nc.sync.dma_start(
                            normsb[b + w : b + 2 * w, :], normsb[b : b + w, :]
                        )

            def emit_norm(c):
                # one op over rows 0..50; rows 16/33/50 multiply into zeroed
                # normsb slots -> finite, and wo has zero rows there
                dstage = dstage_of_chunk.pop(c)
                nc.vector.tensor_mul(
                    scaled[0:64, :], dstage[0:64, :], normsb[0:64, :]
                )

            def emit_proj(c):
                cs, ce = c * CHUNK, (c + 1) * CHUNK
                pj = projp.tile([C, CHUNK], f32, name="pj_ps", tag="proj")
                nc.tensor.matmul(
                    pj[:], lhsT=wo[:], rhs=scaled[:], start=True, stop=True
                )
                osb = osbp.tile([C, CHUNK], f32, name="osb_sb", tag="osb")
                nc.vector.tensor_copy(osb[:], pj[:])
                nc.sync.dma_start(y_d[:, cs:ce], osb[:])

            # seeds: chunk-0 projections + first vT batch
            for u in range(3):
                emit_qk(u, 0)
            emit_vt(0)

            di = 0
            n_slots = n_items + LAG + 3 * EPI_DELAY + 4
            for idx in range(n_slots):
                while deferred and deferred[0][0] <= idx:
                    deferred.pop(0)[1]()
                while di < len(drip) and drip[di][0] <= idx:
                    _, kind, arg = drip[di]
                    di += 1
                    if kind == "qk":
                        emit_qk(*arg)
                    else:
                        emit_vt(arg)
                if idx < n_items:
                    emit_scores(idx)
                av = idx - LAG
                if 0 <= av < n_items and av % 3 == 2:
                    c, g, _ = items[av]
                    emit_attnv_block(c, g)
                    if g == NGRP - 1:
                        emit_denoms(c)
                        deferred.append((idx + EPI_DELAY, lambda c=c: emit_norm(c)))
                        deferred.append(
                            (idx + 2 * EPI_DELAY, lambda c=c: emit_proj(c))
                        )
                        deferred.sort(key=lambda z: z[0])
            while deferred:
                deferred.pop(0)[1]()

    if split_waits:
        _split_multi_waits(nc)
    return nc


def host_prep(inputs, cfg=_FULL):
    """Slice/pack the full problem inputs into per-core input maps."""
    CHUNK = cfg["CHUNK"]
    N = cfg["MT"] * 128
    bf = _bf16np()

    x = np.asarray(inputs["x"], dtype=np.float32)
    B = x.shape[0]
    assert B == 1
    xf = np.ascontiguousarray(x.reshape(C, -1))[:, :N]

    gamma0 = float(np.asarray(inputs["gamma"]).reshape(-1)[0])
    branches = [
        (
            np.asarray(inputs[f"w_qkv_{nm}"], dtype=np.float32),
            np.asarray(inputs[f"b_qkv_{nm}"], dtype=np.float32),
            np.asarray(inputs[f"w_out_{nm}"], dtype=np.float32),
            np.asarray(inputs[f"b_out_{nm}"], dtype=np.float32),
        )
        for nm in ("d", "h", "w")
    ]

    beff_total = np.zeros(C, dtype=np.float64)
    for wqkv, bqkv, wout, bout in branches:
        bv = bqkv[2 * C : 3 * C]
        beff_total += gamma0 * (wout.astype(np.float64) @ bv + bout)
    beff_core = (beff_total / NCORES).astype(np.float32)

    addv = np.zeros((C, 1), dtype=np.float32)
    for r in range(4):
        addv[32 * r + 16, 0] = 1.0
    onesr = np.zeros((C, CHUNK), dtype=np.float32)
    onesr[96, :] = 1.0

    in_maps = []
    for h in range(NCORES):
        m = {
            "x": xf.astype(bf),
            "addv": addv,
            "onesr": onesr,
        }
        wv3 = np.zeros((C, 52), dtype=np.float32)
        wo_stacked = np.zeros((C, 128), dtype=np.float32)
        wo_stacked[96, :] = beff_core
        for u, (wqkv, bqkv, wout, bout) in enumerate(branches):
            wq = wqkv[h * DH : (h + 1) * DH, :]  # (16, 128)
            wk = wqkv[C + h * DH : C + (h + 1) * DH, :]
            wvu = wqkv[2 * C + h * DH : 2 * C + (h + 1) * DH, :]
            bqu = bqkv[h * DH : (h + 1) * DH]
            rvec = wk.T @ bqu  # (128,)

            lqm = np.zeros((C, 128), dtype=np.float32)
            lkm = np.zeros((C, 128), dtype=np.float32)
            for r in range(4):
                lqm[:, 32 * r : 32 * r + 16] = wq.T
                lkm[:, 32 * r : 32 * r + 16] = wk.T
                lkm[:, 32 * r + 16] = rvec
            m[f"lq{u}"] = lqm.astype(bf)
            m[f"lk{u}"] = lkm.astype(bf)

            wv3[:, u * 17 : u * 17 + 16] = wvu.T  # col 16 stays 0 (ones memset)
            wo_stacked[17 * u : 17 * u + 16, :] = (
                gamma0 * wout[:, h * DH : (h + 1) * DH].T
            )
        m["wv3"] = wv3.astype(bf)
        m["wo"] = wo_stacked
        in_maps.append(m)
    return in_maps


def gather(results, inputs, cfg=_FULL):
    x = np.asarray(inputs["x"], dtype=np.float32)
    N = cfg["MT"] * 128
    acc = np.zeros((C, N), dtype=np.float32)
    for r in results:
        acc += r["y"]
    out = acc + x.reshape(C, -1)[:, :N]
    return out.reshape(x.shape).astype(np.float32)


def kernel(**inputs) -> np.ndarray:
    from concourse.bass_utils import run_bass_kernel_spmd

    if "nc" not in _CACHE:
        _CACHE["nc"] = build_nc(_FULL)
    nc = _CACHE["nc"]
    in_maps = host_prep(inputs, _FULL)
    res = run_bass_kernel_spmd(nc, in_maps, list(range(NCORES)))
    return gather(res.results, inputs, _FULL)
